# revision 1
# baseline (speedup 1.0000x reference)
"""Trainium2 Bass kernel for per-edge dot products (GNN DotPredictor).

score[e] = sum_d h[src[e], d] * h[dst[e], d]

h: [100000, 64] f32, src/dst: [1250000] int -> score: [1250000] f32.

Strategy: edge-parallel across 8 NeuronCores; each core gets the full h
table in its HBM plus a shard of 156250 edges. The gather engine of choice
is GPSIMD dma_gather (the production embedding-gather op): it consumes a
wrapped int16 index list and fetches one 256B row per index from HBM at
full DMA-engine parallelism. Since indices are int16 (< 32768) the node
table is viewed as 4 chunks of 32768 rows, and edges are binned host-side
into 16 buckets by the (src_chunk, dst_chunk) pair so every gather in a
bucket uses in-range chunk-local indices. The vector engine multiplies
gathered src/dst rows and does a segmented sum over the 64-wide feature
dim. The host inverts the bucket permutation when unsharding.
"""

import numpy as np

N_NODES = 100000
D = 64
E = 1250000
NCORES = 8
P = 128
CHUNK = 32768           # int16-addressable rows per table chunk
NVAR = 4                # node-table chunks per endpoint
NBUCKET = NVAR * NVAR
E_PC = E // NCORES      # 156250 edges per core

_CACHE = {}


def _build(caps, reps=1):
    """Build + compile the Bass program for bucket capacities `caps`
    (tuple of 16 ints, each a multiple of 128, possibly 0).

    reps > 1 repeats the whole workload (for differential timing)."""
    import concourse.bass as bass  # noqa: F401
    import concourse.mybir as mybir
    import concourse.tile as tile
    from concourse import bacc

    sc = sum(caps) // P        # score columns
    s_tot = sum(caps) // 16    # index columns

    nc = bacc.Bacc(
        "TRN2",
        target_bir_lowering=False,
        debug=False,
        enable_asserts=False,
        num_swdge_queues=4,
    )

    h = nc.dram_tensor("h", [N_NODES, D], mybir.dt.float32, kind="ExternalInput")
    sidx = nc.dram_tensor("sidx", [P, s_tot], mybir.dt.int16, kind="ExternalInput")
    didx = nc.dram_tensor("didx", [P, s_tot], mybir.dt.int16, kind="ExternalInput")
    out = nc.dram_tensor("out", [P, sc], mybir.dt.float32, kind="ExternalOutput")

    with tile.TileContext(nc) as tc:
        with (
            tc.tile_pool(name="idx", bufs=1) as idx_pool,
            tc.tile_pool(name="g", bufs=4) as gpool,
            tc.tile_pool(name="acc", bufs=1) as apool,
        ):
            sidx_t = idx_pool.tile([P, s_tot], mybir.dt.int16, tag="sidx")
            didx_t = idx_pool.tile([P, s_tot], mybir.dt.int16, tag="didx")
            scores = apool.tile([P, sc], mybir.dt.float32, tag="scores")

            nc.sync.dma_start(sidx_t[:], sidx[:])
            nc.sync.dma_start(didx_t[:], didx[:])

            SB = 8192  # max indices per dma_gather (descriptor carveout)
            for _rep in range(reps):
                _emit_body(nc, mybir, caps, h, sidx_t, didx_t, scores, gpool, SB)

            nc.sync.dma_start(out[:], scores[:])

    # Spread gathers across the 4 SWDGE queues for more in-flight DMA.
    # Must follow the SCHEDULED Pool-engine order so each of Tile's 8
    # DMASW sem lanes (assigned round-robin in that order) sees a single
    # queue (8 lanes % 4 queues aligns; ucode locks each sem to a queue).
    cnt = 0
    for blk in nc.m.functions[0].blocks:
        for inst in blk.instructions:
            if isinstance(inst, mybir.InstDMAGatherAnt):
                inst.queue_num = cnt % 4
                cnt += 1
    assert cnt > 0

    nc.compile()
    return nc


def _emit_body(nc, mybir, caps, h, sidx_t, didx_t, scores, gpool, SB):
    col = 0   # score/output column offset
    scol = 0  # index column offset
    if True:
            for b, cap in enumerate(caps):
                if cap == 0:
                    continue
                sv, dv = b // NVAR, b % NVAR
                h_src = h[sv * CHUNK:min((sv + 1) * CHUNK, N_NODES), :]
                h_dst = h[dv * CHUNK:min((dv + 1) * CHUNK, N_NODES), :]
                for off in range(0, cap, SB):
                    n = min(SB, cap - off)
                    bb = n // P     # gathered blocks per partition
                    sb = n // 16    # index columns for this sub-batch
                    gs = gpool.tile([P, SB // P, D], mybir.dt.float32, tag="gs")
                    gd = gpool.tile([P, SB // P, D], mybir.dt.float32, tag="gd")

                    nc.gpsimd.dma_gather(
                        out_ap=gs[:, :bb, :],
                        in_ap=h_src,
                        idxs_ap=sidx_t[:, scol:scol + sb],
                        num_idxs=n,
                        num_idxs_reg=n,
                        elem_size=D,
                        single_packet=False,
                    )
                    nc.gpsimd.dma_gather(
                        out_ap=gd[:, :bb, :],
                        in_ap=h_dst,
                        idxs_ap=didx_t[:, scol:scol + sb],
                        num_idxs=n,
                        num_idxs_reg=n,
                        elem_size=D,
                        single_packet=False,
                    )
                    # in-place product then segmented sum over features
                    gss = gs[:, :bb, :]
                    nc.vector.tensor_tensor(
                        out=gss, in0=gss, in1=gd[:, :bb, :],
                        op=mybir.AluOpType.mult
                    )
                    nc.vector.tensor_reduce(
                        out=scores[:, col:col + bb],
                        in_=gss,
                        axis=mybir.AxisListType.X,
                        op=mybir.AluOpType.add,
                    )
                    col += bb
                    scol += sb


def get_nc(caps, reps=1):
    key = (tuple(caps), reps)
    if key not in _CACHE:
        _CACHE[key] = _build(key[0], reps)
    return _CACHE[key]


def _wrap16(idx16):
    """Linear int16 index list (len % 16 == 0) -> [128, len/16] wrapped tile
    (element i at [i % 16, i // 16], replicated for the 8 Q7 cores)."""
    w = idx16.reshape(-1, 16).T
    return np.tile(w, (8, 1))


def _prepare(src32, dst32):
    """Bin each core's edge shard into 16 (src_chunk, dst_chunk) buckets.

    Returns (caps, in-map fragments per core, scatter positions per core).
    """
    per_core = []
    for i in range(NCORES):
        s = src32[i * E_PC:(i + 1) * E_PC]
        d = dst32[i * E_PC:(i + 1) * E_PC]
        # endpoint variant: node-table chunk
        sv = s >> 15
        dv = d >> 15
        bucket = sv * NVAR + dv
        # secondary sort by src for HBM locality in the src gather stream
        perm = np.lexsort((s, bucket))
        counts = np.bincount(bucket, minlength=NBUCKET)
        per_core.append((s, d, bucket, perm, counts))

    all_counts = np.stack([pc[4] for pc in per_core])
    caps = tuple(int(-(-c // P) * P) for c in all_counts.max(axis=0))
    sc = sum(caps) // P

    col_off = np.concatenate([[0], np.cumsum([c // P for c in caps])])

    frags = []
    for s, d, bucket, perm, counts in per_core:
        ssort = s[perm]
        dsort = d[perm]
        cum = np.concatenate([[0], np.cumsum(counts)])
        sidx = np.zeros(sum(caps), np.int16)
        didx = np.zeros(sum(caps), np.int16)
        # flat HBM position of each sorted edge's score: p*sc + col
        pos = np.empty(E_PC, np.int64)
        off = 0
        for b in range(NBUCKET):
            n = int(counts[b])
            lo, hi = int(cum[b]), int(cum[b + 1])
            sidx[off:off + n] = ssort[lo:hi] & 0x7FFF
            didx[off:off + n] = dsort[lo:hi] & 0x7FFF
            i_local = np.arange(n)
            pos[lo:hi] = (i_local % P) * sc + col_off[b] + i_local // P
            off += caps[b]
        frags.append(
            {
                "sidx": _wrap16(sidx),
                "didx": _wrap16(didx),
                "perm": perm,
                "pos": pos,
            }
        )
    return caps, frags


def run_sharded(h, src, dst, trace=False, **kwargs):
    """Run the SPMD kernel; returns (full_output, BassKernelResults)."""
    from concourse.bass_utils import run_bass_kernel_spmd

    h32 = np.ascontiguousarray(np.asarray(h), dtype=np.float32)
    src32 = np.asarray(src).astype(np.int32)
    dst32 = np.asarray(dst).astype(np.int32)

    caps, frags = _prepare(src32, dst32)
    nc = get_nc(caps)

    in_maps = [
        {"h": h32, "sidx": f["sidx"], "didx": f["didx"]} for f in frags
    ]
    res = run_bass_kernel_spmd(
        nc, in_maps, core_ids=list(range(NCORES)), trace=trace, **kwargs
    )

    full = np.empty(E, np.float32)
    for i, f in enumerate(frags):
        flat = np.asarray(res.results[i]["out"]).reshape(-1)
        shard = np.empty(E_PC, np.float32)
        shard[f["perm"]] = flat[f["pos"]]
        full[i * E_PC:(i + 1) * E_PC] = shard
    return full, res


def kernel(h, src, dst):
    full, _ = run_sharded(h, src, dst, trace=False)
    return full



# revision 7
# speedup vs baseline: 1.5085x; 1.5085x over previous
"""Trainium2 Bass kernel for per-edge dot products (GNN DotPredictor).

score[e] = sum_d h[src[e], d] * h[dst[e], d]

h: [100000, 64] f32, src/dst: [1250000] int -> score: [1250000] f32.

Strategy: edge-parallel across 8 NeuronCores; each core gets the full h
table in its HBM plus a shard of 156250 edges. The gather engine of choice
is GPSIMD dma_gather (the production embedding-gather op): it consumes a
wrapped int16 index list and fetches one row per index from HBM at
full DMA-engine parallelism. Since indices are int16 (< 32768) the node
table is viewed as 4 chunks of 32768 rows, and edges are binned host-side
into 16 buckets by the (src_chunk, dst_chunk) pair so every gather in a
bucket uses in-range chunk-local indices. The vector engine multiplies
gathered src/dst rows and does a segmented sum over the 64-wide feature
dim. The host inverts the bucket permutation when unsharding.

Optionally the table is shipped as bf16 padded to a 256B row stride and
gathered with 128B descriptors (half the HBM bytes of f32).
"""

import numpy as np

N_NODES = 100000
D = 64
E = 1250000
NCORES = 8
P = 128
CHUNK = 32768           # int16-addressable rows per table chunk
NVAR = 4                # node-table chunks per endpoint
NBUCKET = NVAR * NVAR
E_PC = E // NCORES      # 156250 edges per core

# variant knobs (defaults = measured-best configuration)
DEFAULT_OPTS = dict(
    dtype="bf16",        # "f32" | "bf16" (bf16: 128B descriptors, padded table)
    single_packet=False,
    bufs=8,              # gather tile pool depth (bufs//2 iterations in flight)
    sb=4096,             # indices per dma_gather call
    tsb=None,            # staging-tile rows (None: = sb; else multiple of sb,
                         #   several gathers fill one tile before one DVE pass)
    scratch=16384,       # dynamic_dma_scratch_size (SWDGE ring carveout)
    dfeat=64,            # gathered features per row (diagnostic: 32/128 vary
                         #   descriptor bytes at constant descriptor count;
                         #   only 64 (and 128 on the padded bf16 table) are
                         #   numerically correct)
)

_CACHE = {}


def _dma_gather_raw(g, out_ap, in_ap, idxs_ap, num_idxs, elem_size, elem_step,
                    queue_num=0, single_packet=False):
    """dma_gather without the bass-level elem_size%256 restriction.

    The 256B restriction in bass.dma_gather exists for the transpose path
    (XBAR tiles); the non-transpose ucode handles any descriptor length.
    Only the row STRIDE must be a 256B multiple (8-bit stride_bytes_256
    instruction field)."""
    import concourse.mybir as mybir
    from concourse import ap_utils
    from concourse.bass import MemorySpace

    assert idxs_ap.dtype == mybir.dt.int16
    assert in_ap.dtype == out_ap.dtype
    assert in_ap.space == MemorySpace.DRAM
    assert idxs_ap.space == MemorySpace.SBUF
    assert out_ap.space == MemorySpace.SBUF
    assert ap_utils.ap_is_contiguous(in_ap.ap[1:])
    assert ap_utils.ap_is_contiguous(out_ap.ap[1:])
    assert ap_utils.ap_is_contiguous(idxs_ap.ap[1:])
    assert in_ap.ap[-1][1] == out_ap.ap[-1][1] == elem_size
    assert in_ap.ap[0][0] == elem_step
    stride_bytes = elem_step * mybir.dt.size(in_ap.dtype)
    assert stride_bytes % 256 == 0
    stride_bytes_256 = stride_bytes // 256
    assert 0 < stride_bytes_256 < 256

    _in_ap = g.lower_ap_dma(in_ap, for_custom_bir_dma=True)
    _idxs_ap = g.lower_ap(idxs_ap)
    _out_ap = g.lower_ap(out_ap)
    return g.add_instruction(
        mybir.InstDMAGatherAnt(
            name=g.bass.get_next_instruction_name(),
            ins=[*_in_ap, _idxs_ap, g.lower_val_access(g.to_reg(num_idxs))],
            outs=[_out_ap],
            transpose=False,
            num_idxs=num_idxs,
            elem_size=elem_size,
            stride_bytes_256=stride_bytes_256,
            gen_mode=0,
            single_packet=single_packet,
            queue_num=queue_num,
            sbuf_tokens_per_rank=0,
            sbuf_free_dim_per_rank=0,
            sbuf_free_dim_pad_per_rank=0,
            sbuf_byte_offset=0,
        )
    )


def _build(caps, reps=1, opts=None):
    """Build + compile the Bass program for bucket capacities `caps`
    (tuple of 16 ints, each a multiple of 128, possibly 0).

    reps > 1 repeats the whole workload (for differential timing)."""
    import concourse.bass as bass  # noqa: F401
    import concourse.mybir as mybir
    import concourse.tile as tile
    from concourse import bacc

    o = dict(DEFAULT_OPTS, **(opts or {}))
    bf16 = o["dtype"] == "bf16"
    gdt = mybir.dt.bfloat16 if bf16 else mybir.dt.float32
    # table row stride in elements of gdt (256B rows either way)
    estep = 128 if bf16 else 64

    sc = sum(caps) // P        # score columns
    s_tot = sum(caps) // 16    # index columns

    nc = bacc.Bacc(
        "TRN2",
        target_bir_lowering=False,
        debug=False,
        enable_asserts=False,
        num_swdge_queues=4,
        dynamic_dma_scratch_size=o["scratch"],
    )

    h = nc.dram_tensor("h", [N_NODES, estep], gdt, kind="ExternalInput")
    sidx = nc.dram_tensor("sidx", [P, s_tot], mybir.dt.int16, kind="ExternalInput")
    didx = nc.dram_tensor("didx", [P, s_tot], mybir.dt.int16, kind="ExternalInput")
    out = nc.dram_tensor("out", [P, sc], mybir.dt.float32, kind="ExternalOutput")

    with tile.TileContext(nc) as tc:
        with (
            tc.tile_pool(name="idx", bufs=1) as idx_pool,
            tc.tile_pool(name="g", bufs=o["bufs"]) as gpool,
            tc.tile_pool(name="acc", bufs=1) as apool,
        ):
            sidx_t = idx_pool.tile([P, s_tot], mybir.dt.int16, tag="sidx")
            didx_t = idx_pool.tile([P, s_tot], mybir.dt.int16, tag="didx")
            scores = apool.tile([P, sc], mybir.dt.float32, tag="scores")

            nc.sync.dma_start(sidx_t[:], sidx[:])
            nc.sync.dma_start(didx_t[:], didx[:])

            SB = o["sb"]  # max indices per dma_gather (descriptor carveout)
            for _rep in range(reps):
                _emit_body(nc, mybir, caps, h, sidx_t, didx_t, scores, gpool,
                           SB, gdt, estep, o)

            nc.sync.dma_start(out[:], scores[:])

    # Spread gathers across the 4 SWDGE queues for more in-flight DMA.
    # Must follow the SCHEDULED Pool-engine order so each of Tile's 8
    # DMASW sem lanes (assigned round-robin in that order) sees a single
    # queue (8 lanes % 4 queues aligns; ucode locks each sem to a queue).
    cnt = 0
    for blk in nc.m.functions[0].blocks:
        for inst in blk.instructions:
            if isinstance(inst, mybir.InstDMAGatherAnt):
                inst.queue_num = cnt % 4
                cnt += 1
    assert cnt > 0

    nc.compile()
    return nc


def _emit_body(nc, mybir, caps, h, sidx_t, didx_t, scores, gpool, SB,
               gdt, estep, o):
    TSB = o["tsb"] or SB   # staging tile rows (>= SB, multiple of SB)
    assert TSB % SB == 0
    DF = o["dfeat"]
    col = 0   # score/output column offset
    scol = 0  # index column offset
    for b, cap in enumerate(caps):
        if cap == 0:
            continue
        sv, dv = b // NVAR, b % NVAR
        h_src = h[sv * CHUNK:min((sv + 1) * CHUNK, N_NODES), :DF]
        h_dst = h[dv * CHUNK:min((dv + 1) * CHUNK, N_NODES), :DF]
        for off in range(0, cap, TSB):
            nt = min(TSB, cap - off)
            bt = nt // P    # tile blocks per partition
            gs = gpool.tile([P, TSB // P, DF], gdt, tag="gs")
            gd = gpool.tile([P, TSB // P, DF], gdt, tag="gd")

            # fill the tile with several small gathers (finer DMA
            # interleave across the 4 SWDGE queues)
            for q0 in range(0, nt, SB):
                n = min(SB, nt - q0)
                bb = n // P
                sb = n // 16
                b0 = q0 // P
                s0 = scol + q0 // 16
                _dma_gather_raw(
                    nc.gpsimd,
                    out_ap=gs[:, b0:b0 + bb, :],
                    in_ap=h_src,
                    idxs_ap=sidx_t[:, s0:s0 + sb],
                    num_idxs=n,
                    elem_size=DF,
                    elem_step=estep,
                    single_packet=o["single_packet"],
                )
                _dma_gather_raw(
                    nc.gpsimd,
                    out_ap=gd[:, b0:b0 + bb, :],
                    in_ap=h_dst,
                    idxs_ap=didx_t[:, s0:s0 + sb],
                    num_idxs=n,
                    elem_size=DF,
                    elem_step=estep,
                    single_packet=o["single_packet"],
                )
            # in-place product then segmented sum over features
            gss = gs[:, :bt, :]
            nc.vector.tensor_tensor(
                out=gss, in0=gss, in1=gd[:, :bt, :],
                op=mybir.AluOpType.mult
            )
            nc.vector.tensor_reduce(
                out=scores[:, col:col + bt],
                in_=gss,
                axis=mybir.AxisListType.X,
                op=mybir.AluOpType.add,
            )
            col += bt
            scol += nt // 16


def get_nc(caps, reps=1, opts=None):
    o = dict(DEFAULT_OPTS, **(opts or {}))
    key = (tuple(caps), reps, tuple(sorted(o.items())))
    if key not in _CACHE:
        _CACHE[key] = _build(key[0], reps, o)
    return _CACHE[key]


def _wrap16(idx16):
    """Linear int16 index list (len % 16 == 0) -> [128, len/16] wrapped tile
    (element i at [i % 16, i // 16], replicated for the 8 Q7 cores)."""
    w = idx16.reshape(-1, 16).T
    return np.tile(w, (8, 1))


def _prepare(src32, dst32):
    """Bin each core's edge shard into 16 (src_chunk, dst_chunk) buckets.

    Returns (caps, in-map fragments per core, scatter positions per core).
    """
    per_core = []
    for i in range(NCORES):
        s = src32[i * E_PC:(i + 1) * E_PC]
        d = dst32[i * E_PC:(i + 1) * E_PC]
        # endpoint variant: node-table chunk
        sv = s >> 15
        dv = d >> 15
        bucket = sv * NVAR + dv
        # secondary sort by src for HBM locality in the src gather stream
        perm = np.lexsort((s, bucket))
        counts = np.bincount(bucket, minlength=NBUCKET)
        per_core.append((s, d, bucket, perm, counts))

    all_counts = np.stack([pc[4] for pc in per_core])
    caps = tuple(int(-(-c // P) * P) for c in all_counts.max(axis=0))
    sc = sum(caps) // P

    col_off = np.concatenate([[0], np.cumsum([c // P for c in caps])])

    frags = []
    for s, d, bucket, perm, counts in per_core:
        ssort = s[perm]
        dsort = d[perm]
        cum = np.concatenate([[0], np.cumsum(counts)])
        sidx = np.zeros(sum(caps), np.int16)
        didx = np.zeros(sum(caps), np.int16)
        # flat HBM position of each sorted edge's score: p*sc + col
        pos = np.empty(E_PC, np.int64)
        off = 0
        for b in range(NBUCKET):
            n = int(counts[b])
            lo, hi = int(cum[b]), int(cum[b + 1])
            sidx[off:off + n] = ssort[lo:hi] & 0x7FFF
            didx[off:off + n] = dsort[lo:hi] & 0x7FFF
            i_local = np.arange(n)
            pos[lo:hi] = (i_local % P) * sc + col_off[b] + i_local // P
            off += caps[b]
        frags.append(
            {
                "sidx": _wrap16(sidx),
                "didx": _wrap16(didx),
                "perm": perm,
                "pos": pos,
            }
        )
    return caps, frags


def _table(h, opts=None):
    """Host-side table prep: f32 passthrough or bf16 rows padded to 256B."""
    o = dict(DEFAULT_OPTS, **(opts or {}))
    h32 = np.ascontiguousarray(np.asarray(h), dtype=np.float32)
    if o["dtype"] == "f32":
        return h32
    import ml_dtypes

    hp = np.zeros((N_NODES, 128), dtype=ml_dtypes.bfloat16)
    hp[:, :D] = h32.astype(ml_dtypes.bfloat16)
    return hp


def run_sharded(h, src, dst, trace=False, opts=None, **kwargs):
    """Run the SPMD kernel; returns (full_output, BassKernelResults)."""
    from concourse.bass_utils import run_bass_kernel_spmd

    ht = _table(h, opts)
    src32 = np.asarray(src).astype(np.int32)
    dst32 = np.asarray(dst).astype(np.int32)

    caps, frags = _prepare(src32, dst32)
    nc = get_nc(caps, opts=opts)

    in_maps = [
        {"h": ht, "sidx": f["sidx"], "didx": f["didx"]} for f in frags
    ]
    res = run_bass_kernel_spmd(
        nc, in_maps, core_ids=list(range(NCORES)), trace=trace, **kwargs
    )

    full = np.empty(E, np.float32)
    for i, f in enumerate(frags):
        flat = np.asarray(res.results[i]["out"]).reshape(-1)
        shard = np.empty(E_PC, np.float32)
        shard[f["perm"]] = flat[f["pos"]]
        full[i * E_PC:(i + 1) * E_PC] = shard
    return full, res


def kernel(h, src, dst):
    full, _ = run_sharded(h, src, dst, trace=False)
    return full


# revision 10
# speedup vs baseline: 1.6635x; 1.1027x over previous
"""Trainium2 Bass kernel for per-edge dot products (GNN DotPredictor).

score[e] = sum_d h[src[e], d] * h[dst[e], d]

h: [100000, 64] f32, src/dst: [1250000] int -> score: [1250000] f32.

Strategy: edge-parallel across 8 NeuronCores; each core gets the full h
table in its HBM plus a shard of 156250 edges. The gather engine of choice
is GPSIMD dma_gather (the production embedding-gather op): it consumes a
wrapped int16 index list and fetches one row per index from HBM at
full DMA-engine parallelism. Since indices are int16 (< 32768) the node
table is viewed as 4 chunks of 32768 rows, and edges are binned host-side
into 16 buckets by the (src_chunk, dst_chunk) pair so every gather in a
bucket uses in-range chunk-local indices. The vector engine multiplies
gathered src/dst rows and does a segmented sum over the 64-wide feature
dim. The host inverts the bucket permutation when unsharding.

Optionally the table is shipped as bf16 padded to a 256B row stride and
gathered with 128B descriptors (half the HBM bytes of f32).
"""

import numpy as np

N_NODES = 100000
D = 64
E = 1250000
NCORES = 8
P = 128
CHUNK = 32768           # int16-addressable rows per table chunk
NVAR = 4                # node-table chunks per endpoint
NBUCKET = NVAR * NVAR
E_PC = E // NCORES      # 156250 edges per core

# variant knobs (defaults = measured-best configuration)
DEFAULT_OPTS = dict(
    dtype="bf16",        # "f32" | "bf16" (bf16: 128B descriptors, padded table)
    single_packet=False,
    bufs=8,              # gather tile pool depth (bufs//2 iterations in flight)
    sb=3072,             # indices per dma_gather call
    tsb=None,            # staging-tile rows (None: = sb; else multiple of sb,
                         #   several gathers fill one tile before one DVE pass)
    scratch=16384,       # dynamic_dma_scratch_size (SWDGE ring carveout)
    dfeat=64,            # gathered features per row (diagnostic: 32/128 vary
                         #   descriptor bytes at constant descriptor count;
                         #   only 64 (and 128 on the padded bf16 table) are
                         #   numerically correct)
    qpair=False,         # True: queue = (cnt//2)%4 (gs+gd of an iteration on
                         #   one queue) instead of cnt%4
)

_CACHE = {}


def _dma_gather_raw(g, out_ap, in_ap, idxs_ap, num_idxs, elem_size, elem_step,
                    queue_num=0, single_packet=False):
    """dma_gather without the bass-level elem_size%256 restriction.

    The 256B restriction in bass.dma_gather exists for the transpose path
    (XBAR tiles); the non-transpose ucode handles any descriptor length.
    Only the row STRIDE must be a 256B multiple (8-bit stride_bytes_256
    instruction field)."""
    import concourse.mybir as mybir
    from concourse import ap_utils
    from concourse.bass import MemorySpace

    assert idxs_ap.dtype == mybir.dt.int16
    assert in_ap.dtype == out_ap.dtype
    assert in_ap.space == MemorySpace.DRAM
    assert idxs_ap.space == MemorySpace.SBUF
    assert out_ap.space == MemorySpace.SBUF
    assert ap_utils.ap_is_contiguous(in_ap.ap[1:])
    assert ap_utils.ap_is_contiguous(out_ap.ap[1:])
    assert ap_utils.ap_is_contiguous(idxs_ap.ap[1:])
    assert in_ap.ap[-1][1] == out_ap.ap[-1][1] == elem_size
    assert in_ap.ap[0][0] == elem_step
    stride_bytes = elem_step * mybir.dt.size(in_ap.dtype)
    assert stride_bytes % 256 == 0
    stride_bytes_256 = stride_bytes // 256
    assert 0 < stride_bytes_256 < 256

    _in_ap = g.lower_ap_dma(in_ap, for_custom_bir_dma=True)
    _idxs_ap = g.lower_ap(idxs_ap)
    _out_ap = g.lower_ap(out_ap)
    return g.add_instruction(
        mybir.InstDMAGatherAnt(
            name=g.bass.get_next_instruction_name(),
            ins=[*_in_ap, _idxs_ap, g.lower_val_access(g.to_reg(num_idxs))],
            outs=[_out_ap],
            transpose=False,
            num_idxs=num_idxs,
            elem_size=elem_size,
            stride_bytes_256=stride_bytes_256,
            gen_mode=0,
            single_packet=single_packet,
            queue_num=queue_num,
            sbuf_tokens_per_rank=0,
            sbuf_free_dim_per_rank=0,
            sbuf_free_dim_pad_per_rank=0,
            sbuf_byte_offset=0,
        )
    )


def _build(caps, reps=1, opts=None):
    """Build + compile the Bass program for bucket capacities `caps`
    (tuple of 16 ints, each a multiple of 128, possibly 0).

    reps > 1 repeats the whole workload (for differential timing)."""
    import concourse.bass as bass  # noqa: F401
    import concourse.mybir as mybir
    import concourse.tile as tile
    from concourse import bacc

    o = dict(DEFAULT_OPTS, **(opts or {}))
    bf16 = o["dtype"] == "bf16"
    gdt = mybir.dt.bfloat16 if bf16 else mybir.dt.float32
    # table row stride in elements of gdt (256B rows either way)
    estep = 128 if bf16 else 64

    sc = sum(caps) // P        # score columns
    s_tot = sum(caps) // 16    # index columns

    nc = bacc.Bacc(
        "TRN2",
        target_bir_lowering=False,
        debug=False,
        enable_asserts=False,
        num_swdge_queues=4,
        dynamic_dma_scratch_size=o["scratch"],
    )

    h = nc.dram_tensor("h", [N_NODES, estep], gdt, kind="ExternalInput")
    sidx = nc.dram_tensor("sidx", [P, s_tot], mybir.dt.int16, kind="ExternalInput")
    didx = nc.dram_tensor("didx", [P, s_tot], mybir.dt.int16, kind="ExternalInput")
    out = nc.dram_tensor("out", [P, sc], mybir.dt.float32, kind="ExternalOutput")

    with tile.TileContext(nc) as tc:
        with (
            tc.tile_pool(name="idx", bufs=1) as idx_pool,
            tc.tile_pool(name="g", bufs=o["bufs"]) as gpool,
            tc.tile_pool(name="acc", bufs=1) as apool,
        ):
            sidx_t = idx_pool.tile([P, s_tot], mybir.dt.int16, tag="sidx")
            didx_t = idx_pool.tile([P, s_tot], mybir.dt.int16, tag="didx")
            scores = apool.tile([P, sc], mybir.dt.float32, tag="scores")

            nc.sync.dma_start(sidx_t[:], sidx[:])
            nc.sync.dma_start(didx_t[:], didx[:])

            SB = o["sb"]  # max indices per dma_gather (descriptor carveout)
            for _rep in range(reps):
                _emit_body(nc, mybir, caps, h, sidx_t, didx_t, scores, gpool,
                           SB, gdt, estep, o)

            nc.sync.dma_start(out[:], scores[:])

    # Spread gathers across the 4 SWDGE queues for more in-flight DMA.
    # Must follow the SCHEDULED Pool-engine order so each of Tile's 8
    # DMASW sem lanes (assigned round-robin in that order) sees a single
    # queue (8 lanes % 4 queues aligns; ucode locks each sem to a queue).
    cnt = 0
    for blk in nc.m.functions[0].blocks:
        for inst in blk.instructions:
            if isinstance(inst, mybir.InstDMAGatherAnt):
                inst.queue_num = (cnt // 2 if o["qpair"] else cnt) % 4
                cnt += 1
    assert cnt > 0

    nc.compile()
    return nc


def _emit_body(nc, mybir, caps, h, sidx_t, didx_t, scores, gpool, SB,
               gdt, estep, o):
    TSB = o["tsb"] or SB   # staging tile rows (>= SB, multiple of SB)
    assert TSB % SB == 0
    DF = o["dfeat"]
    col = 0   # score/output column offset
    scol = 0  # index column offset
    for b, cap in enumerate(caps):
        if cap == 0:
            continue
        sv, dv = b // NVAR, b % NVAR
        h_src = h[sv * CHUNK:min((sv + 1) * CHUNK, N_NODES), :DF]
        h_dst = h[dv * CHUNK:min((dv + 1) * CHUNK, N_NODES), :DF]
        for off in range(0, cap, TSB):
            nt = min(TSB, cap - off)
            bt = nt // P    # tile blocks per partition
            gs = gpool.tile([P, TSB // P, DF], gdt, tag="gs")
            gd = gpool.tile([P, TSB // P, DF], gdt, tag="gd")

            # fill the tile with several small gathers (finer DMA
            # interleave across the 4 SWDGE queues)
            for q0 in range(0, nt, SB):
                n = min(SB, nt - q0)
                bb = n // P
                sb = n // 16
                b0 = q0 // P
                s0 = scol + q0 // 16
                _dma_gather_raw(
                    nc.gpsimd,
                    out_ap=gs[:, b0:b0 + bb, :],
                    in_ap=h_src,
                    idxs_ap=sidx_t[:, s0:s0 + sb],
                    num_idxs=n,
                    elem_size=DF,
                    elem_step=estep,
                    single_packet=o["single_packet"],
                )
                _dma_gather_raw(
                    nc.gpsimd,
                    out_ap=gd[:, b0:b0 + bb, :],
                    in_ap=h_dst,
                    idxs_ap=didx_t[:, s0:s0 + sb],
                    num_idxs=n,
                    elem_size=DF,
                    elem_step=estep,
                    single_packet=o["single_packet"],
                )
            # in-place product then segmented sum over features
            gss = gs[:, :bt, :]
            nc.vector.tensor_tensor(
                out=gss, in0=gss, in1=gd[:, :bt, :],
                op=mybir.AluOpType.mult
            )
            nc.vector.tensor_reduce(
                out=scores[:, col:col + bt],
                in_=gss,
                axis=mybir.AxisListType.X,
                op=mybir.AluOpType.add,
            )
            col += bt
            scol += nt // 16


def get_nc(caps, reps=1, opts=None):
    o = dict(DEFAULT_OPTS, **(opts or {}))
    key = (tuple(caps), reps, tuple(sorted(o.items())))
    if key not in _CACHE:
        _CACHE[key] = _build(key[0], reps, o)
    return _CACHE[key]


def _wrap16(idx16):
    """Linear int16 index list (len % 16 == 0) -> [128, len/16] wrapped tile
    (element i at [i % 16, i // 16], replicated for the 8 Q7 cores)."""
    w = idx16.reshape(-1, 16).T
    return np.tile(w, (8, 1))


def _prepare(src32, dst32):
    """Bin each core's edge shard into 16 (src_chunk, dst_chunk) buckets.

    Returns (caps, in-map fragments per core, scatter positions per core).
    """
    per_core = []
    for i in range(NCORES):
        s = src32[i * E_PC:(i + 1) * E_PC]
        d = dst32[i * E_PC:(i + 1) * E_PC]
        # endpoint variant: node-table chunk
        sv = s >> 15
        dv = d >> 15
        bucket = sv * NVAR + dv
        # secondary sort by src for HBM locality in the src gather stream
        perm = np.lexsort((s, bucket))
        counts = np.bincount(bucket, minlength=NBUCKET)
        per_core.append((s, d, bucket, perm, counts))

    all_counts = np.stack([pc[4] for pc in per_core])
    caps = tuple(int(-(-c // P) * P) for c in all_counts.max(axis=0))
    sc = sum(caps) // P

    col_off = np.concatenate([[0], np.cumsum([c // P for c in caps])])

    frags = []
    for s, d, bucket, perm, counts in per_core:
        ssort = s[perm]
        dsort = d[perm]
        cum = np.concatenate([[0], np.cumsum(counts)])
        sidx = np.zeros(sum(caps), np.int16)
        didx = np.zeros(sum(caps), np.int16)
        # flat HBM position of each sorted edge's score: p*sc + col
        pos = np.empty(E_PC, np.int64)
        off = 0
        for b in range(NBUCKET):
            n = int(counts[b])
            lo, hi = int(cum[b]), int(cum[b + 1])
            sidx[off:off + n] = ssort[lo:hi] & 0x7FFF
            didx[off:off + n] = dsort[lo:hi] & 0x7FFF
            i_local = np.arange(n)
            pos[lo:hi] = (i_local % P) * sc + col_off[b] + i_local // P
            off += caps[b]
        frags.append(
            {
                "sidx": _wrap16(sidx),
                "didx": _wrap16(didx),
                "perm": perm,
                "pos": pos,
            }
        )
    return caps, frags


def _table(h, opts=None):
    """Host-side table prep: f32 passthrough or bf16 rows padded to 256B."""
    o = dict(DEFAULT_OPTS, **(opts or {}))
    h32 = np.ascontiguousarray(np.asarray(h), dtype=np.float32)
    if o["dtype"] == "f32":
        return h32
    import ml_dtypes

    hp = np.zeros((N_NODES, 128), dtype=ml_dtypes.bfloat16)
    hp[:, :D] = h32.astype(ml_dtypes.bfloat16)
    return hp


def run_sharded(h, src, dst, trace=False, opts=None, **kwargs):
    """Run the SPMD kernel; returns (full_output, BassKernelResults)."""
    from concourse.bass_utils import run_bass_kernel_spmd

    ht = _table(h, opts)
    src32 = np.asarray(src).astype(np.int32)
    dst32 = np.asarray(dst).astype(np.int32)

    caps, frags = _prepare(src32, dst32)
    nc = get_nc(caps, opts=opts)

    in_maps = [
        {"h": ht, "sidx": f["sidx"], "didx": f["didx"]} for f in frags
    ]
    res = run_bass_kernel_spmd(
        nc, in_maps, core_ids=list(range(NCORES)), trace=trace, **kwargs
    )

    full = np.empty(E, np.float32)
    for i, f in enumerate(frags):
        flat = np.asarray(res.results[i]["out"]).reshape(-1)
        shard = np.empty(E_PC, np.float32)
        shard[f["perm"]] = flat[f["pos"]]
        full[i * E_PC:(i + 1) * E_PC] = shard
    return full, res


def kernel(h, src, dst):
    full, _ = run_sharded(h, src, dst, trace=False)
    return full
